# revision 42
# baseline (speedup 1.0000x reference)
"""Multi-head attention (B=2, S=2048, D=1024, H=16) on 8 TRN2 NeuronCores.

Sharding: 2-way data parallel over batch x 4-way tensor parallel over heads
(4 heads = 256 dims per core).  Each core computes, for its (batch, head
group): Q/K/V projections, causal attention, and a partial output
projection (row-sharded Wo).  The host sums the 4 partials per batch and
adds bo.

Device layout notes:
  - All projections produce "head-transposed" activations qh^T/kh^T
    [head_dim, S] so the scores matmul scoresT[t, s] = kh @ qh^T needs no
    on-chip transposes.  V is produced in natural layout [S, head_dim] with
    an appended ones column, so the AV matmul also computes the softmax
    denominator (row 64 of its PSUM output) for free.
  - Scores are bounded (~N(0,1)), so softmax needs no max subtraction:
    attn = exp(s/8) * mask, normalized by the matmul-computed denominator.
  - The mask is handled on the host: each [128 key, 512 query] scoresT
    block gets an active column range [lo, hi) (fully-masked columns are
    never computed) plus optional 128-column multiplicative bf16 mask
    tiles.  Works for any mask; for the causal mask this degenerates to
    one shared triangular tile and ~38% less score work.
  - Softmax normalization: reciprocal_approx_fast on the denominator row,
    partition_broadcast on the (otherwise idle) GpSimd engine, one DVE
    multiply.  PSUM is evacuated promptly so the PE never stalls long
    enough for the HAM clock gate to re-throttle.
"""

import sys

sys.path.insert(0, "/opt/trn_rl_repo")

from contextlib import ExitStack

import ml_dtypes
import numpy as np

B, S, D, H = 2, 2048, 1024, 16
DK = D // H            # 64
NCORE = 8
DPB = 2                # data-parallel ways (batch)
TPG = NCORE // DPB     # 4 head groups
GH = H // TPG          # 4 heads per group
GD = GH * DK           # 256 dims per group
NPAIR = GH // 2        # 2 head pairs per group
SQC = 512              # Sq chunk (matmul moving dim)
SKC = 128              # Skv chunk (matmul partition dim)
MCH = 128              # mask chunk width
NI = S // SQC          # 4
NJ = S // SKC          # 16
KCH = D // 128         # 8 contraction chunks for the projections

TRACE = False
LAST_EXEC_NS = None
LAST_RESULT = None

# feature flags (bisectable on hardware); included in the program cache key
USE_PBCAST = False      # gpsimd.partition_broadcast vs K=1 matmul broadcast
USE_MASK_BCAST = False  # step-0 broadcast AP for the mask mul vs 2 muls
USE_RECIP_FAST = True   # reciprocal_approx_fast vs exact reciprocal

_BF = ml_dtypes.bfloat16
_prog_cache = {}


def _classify_mask(mask_st):
    """mask_st: [S, S] bool indexed [query s, key t].

    Returns (cls, tiles): cls[i][j] is None (skip) or a dict with
      lo, hi : active scoresT column range (multiples of MCH)
      muls   : list of (col_off, tile_idx) 128-col multiplicative masks
    tiles: deduped bf16 [SKC, MCH] tiles in scoresT orientation [t, s].
    """
    cls = [[None] * NJ for _ in range(NI)]
    tiles = []
    keys = {}

    def tile_idx(sub):
        t = np.ascontiguousarray(sub.T)  # [SKC t, MCH s]
        key = t.tobytes()
        if key not in keys:
            keys[key] = len(tiles)
            tiles.append(t.astype(_BF))
        return keys[key]

    for i in range(NI):
        sblk = mask_st[i * SQC : (i + 1) * SQC]
        for j in range(NJ):
            blk = sblk[:, j * SKC : (j + 1) * SKC]  # [SQC s, SKC t]
            any_col = blk.any(axis=1)               # per query col of scoresT
            if not any_col.any():
                continue
            nz = np.nonzero(any_col)[0]
            lo = (int(nz[0]) // MCH) * MCH
            hi = -(-(int(nz[-1]) + 1) // MCH) * MCH
            muls = []
            for c in range(lo, hi, MCH):
                sub = blk[c : c + MCH]              # [MCH s, SKC t]
                if not sub.all():
                    muls.append((c, tile_idx(sub)))
            cls[i][j] = {"lo": lo, "hi": hi, "muls": muls}
    return cls, tiles


def _build(cls, n_mask, with_bias):
    """Build the (SPMD, per-core) Bass program."""
    import concourse.bacc as bacc
    import concourse.tile as tile
    from concourse import mybir

    BF = mybir.dt.bfloat16
    F32 = mybir.dt.float32
    AF = mybir.ActivationFunctionType

    nc = bacc.Bacc("TRN2", target_bir_lowering=False, debug=False)

    xqT = nc.dram_tensor("xqT", [D, S], BF, kind="ExternalInput").ap()
    xkT = nc.dram_tensor("xkT", [D, S], BF, kind="ExternalInput").ap()
    xvT = nc.dram_tensor("xvT", [D, S], BF, kind="ExternalInput").ap()
    # packed weights: [128, KCH*GD], chunk kk at cols [kk*GD, (kk+1)*GD)
    wq_d = nc.dram_tensor("WQ", [128, KCH * GD], BF, kind="ExternalInput").ap()
    wk_d = nc.dram_tensor("WK", [128, KCH * GD], BF, kind="ExternalInput").ap()
    wv_d = nc.dram_tensor("WV", [128, KCH * GD], BF, kind="ExternalInput").ap()
    # packed Wo.T slice: [128, 2*D], chunk kc at cols [kc*D, (kc+1)*D)
    wo_d = nc.dram_tensor("WO", [128, 2 * D], BF, kind="ExternalInput").ap()
    msk_d = None
    if n_mask:
        msk_d = nc.dram_tensor(
            "MSK", [n_mask, SKC, MCH], BF, kind="ExternalInput"
        ).ap()
    if with_bias:
        bq_d = nc.dram_tensor("BQ", [1, GD], BF, kind="ExternalInput").ap()
        bk_d = nc.dram_tensor("BK", [1, GD], BF, kind="ExternalInput").ap()
        bv_d = nc.dram_tensor("BV", [1, GD], BF, kind="ExternalInput").ap()
    y_d = nc.dram_tensor("Y", [S, D], F32, kind="ExternalOutput").ap()

    with tile.TileContext(nc) as tc, ExitStack() as top:
        const = top.enter_context(tc.tile_pool(name="const", bufs=1))

        wq_sb = const.tile([128, KCH * GD], BF, name="wq_sb", tag="wq_sb")
        wk_sb = const.tile([128, KCH * GD], BF, name="wk_sb", tag="wk_sb")
        wv_sb = const.tile([128, KCH * GD], BF, name="wv_sb", tag="wv_sb")
        wo_sb = const.tile([128, 2 * D], BF, name="wo_sb", tag="wo_sb")
        # x inputs fully SBUF-resident: 24 big DMAs (~512KB each), ordered
        # q then k then v so the q-projection can start earliest
        xq_r = [const.tile([128, S], BF, name=f"xq{kk}", tag=f"xq{kk}")
                for kk in range(KCH)]
        xk_r = [const.tile([128, S], BF, name=f"xk{kk}", tag=f"xk{kk}")
                for kk in range(KCH)]
        xv_r = [const.tile([128, S], BF, name=f"xv{kk}", tag=f"xv{kk}")
                for kk in range(KCH)]

        nc.sync.dma_start(out=wq_sb[:], in_=wq_d[:])
        nc.gpsimd.dma_start(out=wk_sb[:], in_=wk_d[:])
        for kk in range(KCH):
            rr = slice(kk * 128, (kk + 1) * 128)
            eng = nc.gpsimd if kk % 2 else nc.sync
            eng.dma_start(out=xq_r[kk][:], in_=xqT[rr, :])
        for kk in range(KCH):
            rr = slice(kk * 128, (kk + 1) * 128)
            eng = nc.sync if kk % 2 else nc.gpsimd
            eng.dma_start(out=xk_r[kk][:], in_=xkT[rr, :])
        nc.sync.dma_start(out=wv_sb[:], in_=wv_d[:])
        for kk in range(KCH):
            rr = slice(kk * 128, (kk + 1) * 128)
            eng = nc.gpsimd if kk % 2 else nc.sync
            eng.dma_start(out=xv_r[kk][:], in_=xvT[rr, :])
        nc.gpsimd.dma_start(out=wo_sb[:], in_=wo_d[:])

        msk_sb = []
        for t in range(n_mask):
            m = const.tile([SKC, MCH], BF, name=f"msk{t}", tag=f"msk{t}")
            nc.sync.dma_start(out=m[:], in_=msk_d[t])
            msk_sb.append(m)

        if with_bias:
            onesrow = const.tile([1, SQC], BF, name="onesrow", tag="onesrow")
            nc.vector.memset(onesrow[:], 1.0)
            bq_sb = const.tile([1, GD], BF, name="bq_sb", tag="bq_sb")
            bk_sb = const.tile([1, GD], BF, name="bk_sb", tag="bk_sb")
            bv_sb = const.tile([1, GD], BF, name="bv_sb", tag="bv_sb")
            nc.sync.dma_start(out=bq_sb[:], in_=bq_d[:])
            nc.sync.dma_start(out=bk_sb[:], in_=bk_d[:])
            nc.sync.dma_start(out=bv_sb[:], in_=bv_d[:])

        # persistent activations
        acts = top.enter_context(tc.tile_pool(name="acts", bufs=1))
        qhT = [acts.tile([128, S], BF, name=f"qhT{p}", tag=f"qhT{p}")
               for p in range(NPAIR)]
        khT = [acts.tile([128, S], BF, name=f"khT{p}", tag=f"khT{p}")
               for p in range(NPAIR)]
        # v in natural layout, 65 cols per head (64 dims + ones column)
        vh = [acts.tile([128, GH * 65], BF, name=f"vh{j}", tag=f"vh{j}")
              for j in range(NJ)]
        for j in range(NJ):
            v3 = vh[j].rearrange("p (h x) -> p h x", h=GH)
            nc.vector.memset(v3[:, :, 64:65], 1.0)

        # ---------------- Phase B: projections ----------------
        with (
            tc.tile_pool(name="pproj", bufs=1, space="PSUM") as pproj,
        ):
            for sc in range(NI):
                psq = [pproj.tile([128, SQC], F32, name=f"psq{m}", tag=f"psq{m}")
                       for m in range(2)]
                psk = [pproj.tile([128, SQC], F32, name=f"psk{m}", tag=f"psk{m}")
                       for m in range(2)]
                psv = [pproj.tile([128, GD], F32, name=f"psv{m}", tag=f"psv{m}")
                       for m in range(4)]
                cc = slice(sc * SQC, (sc + 1) * SQC)
                for kk in range(KCH):
                    st = kk == 0
                    sp = (kk == KCH - 1) and not with_bias
                    for m in range(2):
                        wcol = slice(kk * GD + m * 128, kk * GD + (m + 1) * 128)
                        nc.tensor.matmul(
                            psq[m][:], wq_sb[:, wcol], xq_r[kk][:, cc],
                            start=st, stop=sp,
                        )
                        nc.tensor.matmul(
                            psk[m][:], wk_sb[:, wcol], xk_r[kk][:, cc],
                            start=st, stop=sp,
                        )
                    for m in range(4):
                        nc.tensor.matmul(
                            psv[m][:],
                            xv_r[kk][:, sc * SQC + m * 128 : sc * SQC + (m + 1) * 128],
                            wv_sb[:, kk * GD : (kk + 1) * GD],
                            start=st,
                            stop=sp,
                        )
                if with_bias:
                    for m in range(2):
                        bcol = slice(m * 128, (m + 1) * 128)
                        nc.tensor.matmul(
                            psq[m][:], bq_sb[:, bcol], onesrow[:],
                            start=False, stop=True,
                        )
                        nc.tensor.matmul(
                            psk[m][:], bk_sb[:, bcol], onesrow[:],
                            start=False, stop=True,
                        )
                    for m in range(4):
                        nc.tensor.matmul(
                            psv[m][:], onesrow[:, 0:128], bv_sb[:],
                            start=False, stop=True,
                        )
                for m in range(2):
                    nc.scalar.copy(qhT[m][:, cc], psq[m][:])
                    nc.scalar.copy(khT[m][:, cc], psk[m][:])
                for m in range(4):
                    dst = vh[sc * 4 + m].rearrange("p (h x) -> p h x", h=GH)
                    src = psv[m].rearrange("p (h x) -> p h x", h=GH)
                    nc.vector.tensor_copy(dst[:, :, 0:64], src[:])

        # ---------------- Phase C: attention + Wo ----------------
        # ones on all 128 partitions; single rows are the lhsT of the K=1
        # denominator-broadcast matmuls (lhsT base must match rhs row base)
        onesP = const.tile([128, 64], BF, name="onesP", tag="onesP")
        nc.vector.memset(onesP[:], 1.0)

        with (
            tc.tile_pool(name="psc", bufs=2, space="PSUM") as psc,
            tc.tile_pool(name="pso", bufs=1, space="PSUM") as pso,
            tc.tile_pool(name="psy", bufs=2, space="PSUM") as psy,
            tc.tile_pool(name="ex", bufs=3) as expool,
            tc.tile_pool(name="nrm", bufs=2) as nrm,
            tc.tile_pool(name="aou", bufs=8) as aoupool,
            tc.tile_pool(name="ao", bufs=2) as aopool,
            tc.tile_pool(name="yout", bufs=3) as ypool,
        ):
            def emit_attention(i, fillers=None):
                """scores/exp/mask/AV + psO evacuation + reciprocal chain.
                `fillers`: deferred norm/Wo closures from the previous chunk,
                emitted one per j-iteration to fill the PE's exp-wait idle."""
                fillers = list(fillers or [])
                js = [j for j in range(NJ) if cls[i][j] is not None]
                assert js, "fully-masked query chunk not supported"
                aoT = [
                    aopool.tile([128, SQC], BF, name=f"aoT{p}", tag=f"aoT{p}")
                    for p in range(NPAIR)
                ]
                aoUs = []
                for p in range(NPAIR):
                    psO = [
                        pso.tile([65, SQC], F32, name=f"psO{h}", tag=f"psO{h}")
                        for h in range(2)
                    ]

                    def emit_av(av):
                        jn, j, lo, hi, e = av
                        for h in range(2):
                            vcol = slice((2 * p + h) * 65, (2 * p + h + 1) * 65)
                            nc.tensor.matmul(
                                psO[h][:, lo:hi],
                                vh[j][:, vcol],
                                e[:, h * SQC + lo : h * SQC + hi],
                                start=(jn == 0), stop=(jn == len(js) - 1),
                            )

                    # AV matmuls are emitted one j behind the scores matmuls:
                    # the in-order PE can then run scores_{j+1} while the ACT
                    # engine computes exp_j, instead of stalling on it.
                    pend_av = None
                    for jn, j in enumerate(js):
                        c = cls[i][j]
                        lo, hi = c["lo"], c["hi"]
                        jw = slice(j * SKC, (j + 1) * SKC)
                        iw = slice(i * SQC + lo, i * SQC + hi)
                        # h0 in cols [0:SQC], h1 in cols [SQC:2*SQC]
                        ps = psc.tile([128, 2 * SQC], F32, name="ps", tag="ps")
                        e = expool.tile([128, 2 * SQC], BF, name="e", tag="e")
                        for h in range(2):
                            pr = slice(h * 64, (h + 1) * 64)
                            nc.tensor.matmul(
                                ps[:, h * SQC + lo : h * SQC + hi],
                                khT[p][pr, jw],
                                qhT[p][pr, iw],
                                start=True, stop=True,
                            )
                        ps3 = ps.rearrange("p (h c) -> p h c", h=2)
                        e3 = e.rearrange("p (h c) -> p h c", h=2)
                        nc.scalar.activation(
                            e3[:, :, lo:hi], ps3[:, :, lo:hi], AF.Exp,
                            scale=1.0 / np.sqrt(DK),
                        )
                        for c0, tidx in c["muls"]:
                            for h in range(2):
                                cw = slice(h * SQC + c0, h * SQC + c0 + MCH)
                                nc.vector.tensor_mul(
                                    e[:, cw], e[:, cw], msk_sb[tidx][:]
                                )
                        if pend_av is not None:
                            emit_av(pend_av)
                        if fillers and (p > 0 or jn >= 2):
                            fillers.pop(0)()
                        pend_av = (jn, j, lo, hi, e)
                    emit_av(pend_av)
                    # evacuate promptly (frees the psO banks); row 64 is the
                    # softmax denominator
                    for h in range(2):
                        aoU = aoupool.tile([65, SQC], F32, name="aoU", tag="aoU")
                        nc.vector.tensor_copy(aoU[:], psO[h][:])
                        aoUs.append(aoU)
                for f in fillers:
                    f()
                del fillers[:]
                # gather the 4 denominator rows onto partitions {0,32,64,96}
                # (tiny SBUF->SBUF DMAs), ONE reciprocal for all heads
                den_t = nrm.tile([97, SQC], F32, name="den_t", tag="den_t")
                nc.vector.memset(den_t[:], 1.0)
                for idx, aoU in enumerate(aoUs):
                    eng = nc.gpsimd if idx % 2 else nc.sync
                    eng.dma_start(
                        out=den_t[32 * idx : 32 * idx + 1, :], in_=aoU[64:65, :]
                    )
                rc_t = nrm.tile([97, SQC], F32, name="rc_t", tag="rc_t")
                nc.vector.reciprocal(rc_t[:], den_t[:])
                rcb_t = nrm.tile([97, SQC], BF, name="rcb_t", tag="rcb_t")
                nc.vector.tensor_copy(rcb_t[:], rc_t[:])
                return i, aoT, aoUs, rcb_t

            def make_norm_wo(state):
                """Deferred broadcast + normalize + Wo closures for a
                finished i, to be interleaved into the next chunk."""
                i, aoT, aoUs, rcb_t = state
                bcd = nrm.tile([64, 4 * SQC], F32, name="bcd", tag="bcd")
                fillers = []

                def mk_bcast(idx):
                    def f():
                        r = 32 * idx
                        psB = psy.tile([64, SQC], F32, name="psB", tag="pY")
                        nc.tensor.matmul(
                            psB[:],
                            onesP[r : r + 1, :],
                            rcb_t[r : r + 1, :],
                            start=True, stop=True,
                            tile_position=(r, 0),
                        )
                        nc.vector.tensor_copy(
                            bcd[:, idx * SQC : (idx + 1) * SQC], psB[:]
                        )
                    return f

                for idx in range(4):
                    fillers.append(mk_bcast(idx))

                def muls():
                    for p in range(NPAIR):
                        for h in range(2):
                            idx = 2 * p + h
                            nc.vector.tensor_mul(
                                aoT[p][h * 64 : (h + 1) * 64, :],
                                aoUs[idx][0:64, :],
                                bcd[:, idx * SQC : (idx + 1) * SQC],
                            )
                fillers.append(muls)

                def mk_wo(m, n):
                    def f():
                        rw = slice(m * 128, (m + 1) * 128)
                        orows = slice(
                            i * SQC + m * 128, i * SQC + (m + 1) * 128
                        )
                        ncol = slice(n * SQC, (n + 1) * SQC)
                        pY = psy.tile([128, SQC], F32, name="pY", tag="pY")
                        for kc in range(NPAIR):
                            nc.tensor.matmul(
                                pY[:],
                                aoT[kc][:, rw],
                                wo_sb[:, kc * D + n * SQC : kc * D + (n + 1) * SQC],
                                start=(kc == 0),
                                stop=(kc == NPAIR - 1),
                            )
                        y_sb = ypool.tile(
                            [128, SQC], F32, name="y_sb", tag="y_sb"
                        )
                        nc.vector.tensor_copy(y_sb[:], pY[:])
                        nc.sync.dma_start(out=y_d[orows, ncol], in_=y_sb[:])
                    return f

                for m in range(4):
                    for n in range(2):
                        fillers.append(mk_wo(m, n))
                return fillers

            pending = None
            for i in range(NI):
                fillers = make_norm_wo(pending) if pending is not None else None
                pending = emit_attention(i, fillers)
            for f in make_norm_wo(pending):
                f()

    nc.compile()
    return nc


def _cls_sig(cls):
    out = []
    for row in cls:
        for c in row:
            if c is None:
                out.append(None)
            else:
                out.append((c["lo"], c["hi"], tuple(c["muls"])))
    return tuple(out)


def kernel(q, k, v, Wq, bq, Wk, bk, Wv, bv, Wo, bo, mask):
    global LAST_EXEC_NS, LAST_RESULT
    from concourse.bass_utils import run_bass_kernel_spmd

    q = np.asarray(q, np.float32)
    k = np.asarray(k, np.float32)
    v = np.asarray(v, np.float32)
    mask_st = np.asarray(mask).reshape(S, S).astype(bool)

    cls, mtiles = _classify_mask(mask_st)
    with_bias = not (
        np.all(np.asarray(bq) == 0)
        and np.all(np.asarray(bk) == 0)
        and np.all(np.asarray(bv) == 0)
    )

    sig = (
        _cls_sig(cls), len(mtiles), with_bias,
        USE_PBCAST, USE_MASK_BCAST, USE_RECIP_FAST,
    )
    if sig not in _prog_cache:
        _prog_cache[sig] = _build(cls, len(mtiles), with_bias)
    nc = _prog_cache[sig]

    def pack_w(wt, gd):  # [nch*128, gd] -> [128, nch*gd]
        nch = wt.shape[0] // 128
        return np.ascontiguousarray(
            wt.reshape(nch, 128, gd).transpose(1, 0, 2).reshape(128, nch * gd)
        ).astype(_BF)

    in_maps = []
    for c in range(NCORE):
        b, g = divmod(c, TPG)
        rows = slice(g * GD, (g + 1) * GD)
        im = {
            "xqT": np.ascontiguousarray(q[b].T).astype(_BF),
            "xkT": np.ascontiguousarray(k[b].T).astype(_BF),
            "xvT": np.ascontiguousarray(v[b].T).astype(_BF),
            "WQ": pack_w(np.ascontiguousarray(Wq[rows, :].T), GD),
            "WK": pack_w(np.ascontiguousarray(Wk[rows, :].T), GD),
            "WV": pack_w(np.ascontiguousarray(Wv[rows, :].T), GD),
            "WO": pack_w(np.ascontiguousarray(Wo[:, rows].T), D),
        }
        if mtiles:
            im["MSK"] = np.stack(mtiles)
        if with_bias:
            im["BQ"] = np.asarray(bq)[rows].reshape(1, GD).astype(_BF)
            im["BK"] = np.asarray(bk)[rows].reshape(1, GD).astype(_BF)
            im["BV"] = np.asarray(bv)[rows].reshape(1, GD).astype(_BF)
        in_maps.append(im)

    res = run_bass_kernel_spmd(nc, in_maps, list(range(NCORE)), trace=TRACE)
    LAST_RESULT = res
    LAST_EXEC_NS = res.exec_time_ns

    out = np.zeros((B, S, D), np.float32)
    for c in range(NCORE):
        out[c // TPG] += res.results[c]["Y"]
    out += np.asarray(bo, np.float32)
    return out


# revision 43
# speedup vs baseline: 1.0303x; 1.0303x over previous
"""Multi-head attention (B=2, S=2048, D=1024, H=16) on 8 TRN2 NeuronCores.

Sharding: 2-way data parallel over batch x 4-way tensor parallel over heads
(4 heads = 256 dims per core).  Each core computes, for its (batch, head
group): Q/K/V projections, causal attention, and a partial output
projection (row-sharded Wo).  The host sums the 4 partials per batch and
adds bo.

Device layout notes:
  - All projections produce "head-transposed" activations qh^T/kh^T
    [head_dim, S] so the scores matmul scoresT[t, s] = kh @ qh^T needs no
    on-chip transposes.  V is produced in natural layout [S, head_dim] with
    an appended ones column, so the AV matmul also computes the softmax
    denominator (row 64 of its PSUM output) for free.
  - Scores are bounded (~N(0,1)), so softmax needs no max subtraction:
    attn = exp(s/8) * mask, normalized by the matmul-computed denominator.
  - The mask is handled on the host: each [128 key, 512 query] scoresT
    block gets an active column range [lo, hi) (fully-masked columns are
    never computed) plus optional 128-column multiplicative bf16 mask
    tiles.  Works for any mask; for the causal mask this degenerates to
    one shared triangular tile and ~38% less score work.
  - Softmax normalization: the 4 denominator rows are DMA-gathered onto
    partitions {0,32,64,96}, one reciprocal serves all heads, and K=1 bf16
    matmuls broadcast each row across 64 partitions.  The whole chain and
    the Wo projection are emitted one chunk late, so the in-order PE never
    waits on it (long PE stalls re-throttle the HAM clock gate to 1.2GHz).
  - AV matmuls are emitted one j-iteration behind their scores matmuls so
    the PE overlaps the ACT engine's exp instead of stalling on it.
"""

import sys

sys.path.insert(0, "/opt/trn_rl_repo")

from contextlib import ExitStack

import ml_dtypes
import numpy as np

B, S, D, H = 2, 2048, 1024, 16
DK = D // H            # 64
NCORE = 8
DPB = 2                # data-parallel ways (batch)
TPG = NCORE // DPB     # 4 head groups
GH = H // TPG          # 4 heads per group
GD = GH * DK           # 256 dims per group
NPAIR = GH // 2        # 2 head pairs per group
SQC = 512              # Sq chunk (matmul moving dim)
SKC = 128              # Skv chunk (matmul partition dim)
MCH = 128              # mask chunk width
NI = S // SQC          # 4
NJ = S // SKC          # 16
KCH = D // 128         # 8 contraction chunks for the projections

TRACE = False
LAST_EXEC_NS = None
LAST_RESULT = None

_BF = ml_dtypes.bfloat16
_prog_cache = {}


def _classify_mask(mask_st):
    """mask_st: [S, S] bool indexed [query s, key t].

    Returns (cls, tiles): cls[i][j] is None (skip) or a dict with
      lo, hi : active scoresT column range (multiples of MCH)
      muls   : list of (col_off, tile_idx) 128-col multiplicative masks
    tiles: deduped bf16 [SKC, MCH] tiles in scoresT orientation [t, s].
    """
    cls = [[None] * NJ for _ in range(NI)]
    tiles = []
    keys = {}

    def tile_idx(sub):
        t = np.ascontiguousarray(sub.T)  # [SKC t, MCH s]
        key = t.tobytes()
        if key not in keys:
            keys[key] = len(tiles)
            tiles.append(t.astype(_BF))
        return keys[key]

    for i in range(NI):
        sblk = mask_st[i * SQC : (i + 1) * SQC]
        for j in range(NJ):
            blk = sblk[:, j * SKC : (j + 1) * SKC]  # [SQC s, SKC t]
            any_col = blk.any(axis=1)               # per query col of scoresT
            if not any_col.any():
                continue
            nz = np.nonzero(any_col)[0]
            lo = (int(nz[0]) // MCH) * MCH
            hi = -(-(int(nz[-1]) + 1) // MCH) * MCH
            muls = []
            for c in range(lo, hi, MCH):
                sub = blk[c : c + MCH]              # [MCH s, SKC t]
                if not sub.all():
                    muls.append((c, tile_idx(sub)))
            cls[i][j] = {"lo": lo, "hi": hi, "muls": muls}
    return cls, tiles


def _build(cls, n_mask, with_bias):
    """Build the (SPMD, per-core) Bass program."""
    import concourse.bacc as bacc
    import concourse.tile as tile
    from concourse import mybir

    BF = mybir.dt.bfloat16
    F32 = mybir.dt.float32
    AF = mybir.ActivationFunctionType

    nc = bacc.Bacc("TRN2", target_bir_lowering=False, debug=False)

    xqT = nc.dram_tensor("xqT", [D, S], BF, kind="ExternalInput").ap()
    xkT = nc.dram_tensor("xkT", [D, S], BF, kind="ExternalInput").ap()
    xvT = nc.dram_tensor("xvT", [D, S], BF, kind="ExternalInput").ap()
    # packed weights: [128, KCH*GD], chunk kk at cols [kk*GD, (kk+1)*GD)
    wq_d = nc.dram_tensor("WQ", [128, KCH * GD], BF, kind="ExternalInput").ap()
    wk_d = nc.dram_tensor("WK", [128, KCH * GD], BF, kind="ExternalInput").ap()
    wv_d = nc.dram_tensor("WV", [128, KCH * GD], BF, kind="ExternalInput").ap()
    # packed Wo.T slice: [128, 2*D], chunk kc at cols [kc*D, (kc+1)*D)
    wo_d = nc.dram_tensor("WO", [128, 2 * D], BF, kind="ExternalInput").ap()
    msk_d = None
    if n_mask:
        msk_d = nc.dram_tensor(
            "MSK", [n_mask, SKC, MCH], BF, kind="ExternalInput"
        ).ap()
    if with_bias:
        bq_d = nc.dram_tensor("BQ", [1, GD], BF, kind="ExternalInput").ap()
        bk_d = nc.dram_tensor("BK", [1, GD], BF, kind="ExternalInput").ap()
        bv_d = nc.dram_tensor("BV", [1, GD], BF, kind="ExternalInput").ap()
    y_d = nc.dram_tensor("Y", [S, D], F32, kind="ExternalOutput").ap()

    with tile.TileContext(nc) as tc, ExitStack() as top:
        const = top.enter_context(tc.tile_pool(name="const", bufs=1))

        wq_sb = const.tile([128, KCH * GD], BF, name="wq_sb", tag="wq_sb")
        wk_sb = const.tile([128, KCH * GD], BF, name="wk_sb", tag="wk_sb")
        wv_sb = const.tile([128, KCH * GD], BF, name="wv_sb", tag="wv_sb")
        wo_sb = const.tile([128, 2 * D], BF, name="wo_sb", tag="wo_sb")
        # x inputs fully SBUF-resident: 24 big DMAs (~512KB each), ordered
        # q then k then v so the q-projection can start earliest
        xq_r = [const.tile([128, S], BF, name=f"xq{kk}", tag=f"xq{kk}")
                for kk in range(KCH)]
        xk_r = [const.tile([128, S], BF, name=f"xk{kk}", tag=f"xk{kk}")
                for kk in range(KCH)]
        xv_r = [const.tile([128, S], BF, name=f"xv{kk}", tag=f"xv{kk}")
                for kk in range(KCH)]

        nc.sync.dma_start(out=wq_sb[:], in_=wq_d[:])
        nc.gpsimd.dma_start(out=wk_sb[:], in_=wk_d[:])
        for kk in range(KCH):
            rr = slice(kk * 128, (kk + 1) * 128)
            eng = nc.gpsimd if kk % 2 else nc.sync
            eng.dma_start(out=xq_r[kk][:], in_=xqT[rr, :])
        for kk in range(KCH):
            rr = slice(kk * 128, (kk + 1) * 128)
            eng = nc.sync if kk % 2 else nc.gpsimd
            eng.dma_start(out=xk_r[kk][:], in_=xkT[rr, :])
        nc.sync.dma_start(out=wv_sb[:], in_=wv_d[:])
        for kk in range(KCH):
            rr = slice(kk * 128, (kk + 1) * 128)
            eng = nc.gpsimd if kk % 2 else nc.sync
            eng.dma_start(out=xv_r[kk][:], in_=xvT[rr, :])
        nc.gpsimd.dma_start(out=wo_sb[:], in_=wo_d[:])

        msk_sb = []
        for t in range(n_mask):
            m = const.tile([SKC, MCH], BF, name=f"msk{t}", tag=f"msk{t}")
            nc.sync.dma_start(out=m[:], in_=msk_d[t])
            msk_sb.append(m)

        if with_bias:
            onesrow = const.tile([1, SQC], BF, name="onesrow", tag="onesrow")
            nc.vector.memset(onesrow[:], 1.0)
            bq_sb = const.tile([1, GD], BF, name="bq_sb", tag="bq_sb")
            bk_sb = const.tile([1, GD], BF, name="bk_sb", tag="bk_sb")
            bv_sb = const.tile([1, GD], BF, name="bv_sb", tag="bv_sb")
            nc.sync.dma_start(out=bq_sb[:], in_=bq_d[:])
            nc.sync.dma_start(out=bk_sb[:], in_=bk_d[:])
            nc.sync.dma_start(out=bv_sb[:], in_=bv_d[:])

        # persistent activations
        acts = top.enter_context(tc.tile_pool(name="acts", bufs=1))
        qhT = [acts.tile([128, S], BF, name=f"qhT{p}", tag=f"qhT{p}")
               for p in range(NPAIR)]
        khT = [acts.tile([128, S], BF, name=f"khT{p}", tag=f"khT{p}")
               for p in range(NPAIR)]
        # v in natural layout, 65 cols per head (64 dims + ones column)
        vh = [acts.tile([128, GH * 65], BF, name=f"vh{j}", tag=f"vh{j}")
              for j in range(NJ)]
        for j in range(NJ):
            v3 = vh[j].rearrange("p (h x) -> p h x", h=GH)
            nc.vector.memset(v3[:, :, 64:65], 1.0)

        # ---------------- Phase B: projections ----------------
        with (
            tc.tile_pool(name="pproj", bufs=1, space="PSUM") as pproj,
        ):
            for sc in range(NI):
                psq = [pproj.tile([128, SQC], F32, name=f"psq{m}", tag=f"psq{m}")
                       for m in range(2)]
                psk = [pproj.tile([128, SQC], F32, name=f"psk{m}", tag=f"psk{m}")
                       for m in range(2)]
                psv = [pproj.tile([128, GD], F32, name=f"psv{m}", tag=f"psv{m}")
                       for m in range(4)]
                cc = slice(sc * SQC, (sc + 1) * SQC)
                for kk in range(KCH):
                    st = kk == 0
                    sp = (kk == KCH - 1) and not with_bias
                    for m in range(2):
                        wcol = slice(kk * GD + m * 128, kk * GD + (m + 1) * 128)
                        nc.tensor.matmul(
                            psq[m][:], wq_sb[:, wcol], xq_r[kk][:, cc],
                            start=st, stop=sp,
                        )
                        nc.tensor.matmul(
                            psk[m][:], wk_sb[:, wcol], xk_r[kk][:, cc],
                            start=st, stop=sp,
                        )
                    for m in range(4):
                        nc.tensor.matmul(
                            psv[m][:],
                            xv_r[kk][:, sc * SQC + m * 128 : sc * SQC + (m + 1) * 128],
                            wv_sb[:, kk * GD : (kk + 1) * GD],
                            start=st,
                            stop=sp,
                        )
                if with_bias:
                    for m in range(2):
                        bcol = slice(m * 128, (m + 1) * 128)
                        nc.tensor.matmul(
                            psq[m][:], bq_sb[:, bcol], onesrow[:],
                            start=False, stop=True,
                        )
                        nc.tensor.matmul(
                            psk[m][:], bk_sb[:, bcol], onesrow[:],
                            start=False, stop=True,
                        )
                    for m in range(4):
                        nc.tensor.matmul(
                            psv[m][:], onesrow[:, 0:128], bv_sb[:],
                            start=False, stop=True,
                        )
                for m in range(2):
                    nc.scalar.copy(qhT[m][:, cc], psq[m][:])
                    nc.scalar.copy(khT[m][:, cc], psk[m][:])
                for m in range(4):
                    dst = vh[sc * 4 + m].rearrange("p (h x) -> p h x", h=GH)
                    src = psv[m].rearrange("p (h x) -> p h x", h=GH)
                    nc.vector.tensor_copy(dst[:, :, 0:64], src[:])

        # ---------------- Phase C: attention + Wo ----------------
        # ones on all 128 partitions; single rows are the lhsT of the K=1
        # denominator-broadcast matmuls (lhsT base must match rhs row base)
        onesP = const.tile([128, 64], BF, name="onesP", tag="onesP")
        nc.vector.memset(onesP[:], 1.0)

        with (
            tc.tile_pool(name="psc", bufs=2, space="PSUM") as psc,
            tc.tile_pool(name="pso", bufs=1, space="PSUM") as pso,
            tc.tile_pool(name="psy", bufs=2, space="PSUM") as psy,
            tc.tile_pool(name="ex", bufs=3) as expool,
            tc.tile_pool(name="nrm", bufs=2) as nrm,
            tc.tile_pool(name="aou", bufs=8) as aoupool,
            tc.tile_pool(name="ao", bufs=2) as aopool,
            tc.tile_pool(name="yout", bufs=3) as ypool,
        ):
            def emit_attention(i):
                """scores/exp/mask/AV + psO evacuation + reciprocal chain."""
                js = [j for j in range(NJ) if cls[i][j] is not None]
                assert js, "fully-masked query chunk not supported"
                aoT = [
                    aopool.tile([128, SQC], BF, name=f"aoT{p}", tag=f"aoT{p}")
                    for p in range(NPAIR)
                ]
                aoUs = []
                for p in range(NPAIR):
                    psO = [
                        pso.tile([65, SQC], F32, name=f"psO{h}", tag=f"psO{h}")
                        for h in range(2)
                    ]

                    def emit_av(av):
                        jn, j, lo, hi, e = av
                        for h in range(2):
                            vcol = slice((2 * p + h) * 65, (2 * p + h + 1) * 65)
                            nc.tensor.matmul(
                                psO[h][:, lo:hi],
                                vh[j][:, vcol],
                                e[:, h * SQC + lo : h * SQC + hi],
                                start=(jn == 0), stop=(jn == len(js) - 1),
                            )

                    # AV matmuls are emitted one j behind the scores matmuls:
                    # the in-order PE can then run scores_{j+1} while the ACT
                    # engine computes exp_j, instead of stalling on it.
                    pend_av = None
                    for jn, j in enumerate(js):
                        c = cls[i][j]
                        lo, hi = c["lo"], c["hi"]
                        jw = slice(j * SKC, (j + 1) * SKC)
                        iw = slice(i * SQC + lo, i * SQC + hi)
                        # h0 in cols [0:SQC], h1 in cols [SQC:2*SQC]
                        ps = psc.tile([128, 2 * SQC], F32, name="ps", tag="ps")
                        e = expool.tile([128, 2 * SQC], BF, name="e", tag="e")
                        for h in range(2):
                            pr = slice(h * 64, (h + 1) * 64)
                            nc.tensor.matmul(
                                ps[:, h * SQC + lo : h * SQC + hi],
                                khT[p][pr, jw],
                                qhT[p][pr, iw],
                                start=True, stop=True,
                            )
                        ps3 = ps.rearrange("p (h c) -> p h c", h=2)
                        e3 = e.rearrange("p (h c) -> p h c", h=2)
                        nc.scalar.activation(
                            e3[:, :, lo:hi], ps3[:, :, lo:hi], AF.Exp,
                            scale=1.0 / np.sqrt(DK),
                        )
                        for c0, tidx in c["muls"]:
                            for h in range(2):
                                cw = slice(h * SQC + c0, h * SQC + c0 + MCH)
                                nc.vector.tensor_mul(
                                    e[:, cw], e[:, cw], msk_sb[tidx][:]
                                )
                        if pend_av is not None:
                            emit_av(pend_av)
                        pend_av = (jn, j, lo, hi, e)
                    emit_av(pend_av)
                    # evacuate promptly (frees the psO banks); row 64 is the
                    # softmax denominator
                    for h in range(2):
                        aoU = aoupool.tile([65, SQC], F32, name="aoU", tag="aoU")
                        nc.vector.tensor_copy(aoU[:], psO[h][:])
                        aoUs.append(aoU)
                # gather the 4 denominator rows onto partitions {0,32,64,96}
                # (tiny SBUF->SBUF DMAs), ONE reciprocal for all heads
                den_t = nrm.tile([97, SQC], F32, name="den_t", tag="den_t")
                nc.vector.memset(den_t[:], 1.0)
                for idx, aoU in enumerate(aoUs):
                    eng = nc.gpsimd if idx % 2 else nc.sync
                    eng.dma_start(
                        out=den_t[32 * idx : 32 * idx + 1, :], in_=aoU[64:65, :]
                    )
                rc_t = nrm.tile([97, SQC], F32, name="rc_t", tag="rc_t")
                nc.vector.reciprocal(rc_t[:], den_t[:])
                rcb_t = nrm.tile([97, SQC], BF, name="rcb_t", tag="rcb_t")
                nc.vector.tensor_copy(rcb_t[:], rc_t[:])
                return i, aoT, aoUs, rcb_t

            def emit_norm_wo(state):
                """Broadcast + normalize + Wo for a finished i (deferred one
                chunk so the PE never waits on the reciprocal chain)."""
                i, aoT, aoUs, rcb_t = state
                bcd = nrm.tile([64, 4 * SQC], F32, name="bcd", tag="bcd")
                for idx in range(4):
                    r = 32 * idx
                    psB = psy.tile([64, SQC], F32, name="psB", tag="pY")
                    nc.tensor.matmul(
                        psB[:],
                        onesP[r : r + 1, :],
                        rcb_t[r : r + 1, :],
                        start=True, stop=True,
                        tile_position=(r, 0),
                    )
                    nc.vector.tensor_copy(
                        bcd[:, idx * SQC : (idx + 1) * SQC], psB[:]
                    )
                for p in range(NPAIR):
                    for h in range(2):
                        idx = 2 * p + h
                        nc.vector.tensor_mul(
                            aoT[p][h * 64 : (h + 1) * 64, :],
                            aoUs[idx][0:64, :],
                            bcd[:, idx * SQC : (idx + 1) * SQC],
                        )
                for m in range(4):
                    rw = slice(m * 128, (m + 1) * 128)
                    orows = slice(i * SQC + m * 128, i * SQC + (m + 1) * 128)
                    for n in range(2):
                        ncol = slice(n * SQC, (n + 1) * SQC)
                        pY = psy.tile([128, SQC], F32, name="pY", tag="pY")
                        for kc in range(NPAIR):
                            nc.tensor.matmul(
                                pY[:],
                                aoT[kc][:, rw],
                                wo_sb[:, kc * D + n * SQC : kc * D + (n + 1) * SQC],
                                start=(kc == 0),
                                stop=(kc == NPAIR - 1),
                            )
                        y_sb = ypool.tile([128, SQC], F32, name="y_sb", tag="y_sb")
                        nc.vector.tensor_copy(y_sb[:], pY[:])
                        nc.sync.dma_start(out=y_d[orows, ncol], in_=y_sb[:])

            pending = None
            for i in range(NI):
                st = emit_attention(i)
                if pending is not None:
                    emit_norm_wo(pending)
                pending = st
            emit_norm_wo(pending)

    nc.compile()
    return nc


def _cls_sig(cls):
    out = []
    for row in cls:
        for c in row:
            if c is None:
                out.append(None)
            else:
                out.append((c["lo"], c["hi"], tuple(c["muls"])))
    return tuple(out)


def kernel(q, k, v, Wq, bq, Wk, bk, Wv, bv, Wo, bo, mask):
    global LAST_EXEC_NS, LAST_RESULT
    from concourse.bass_utils import run_bass_kernel_spmd

    q = np.asarray(q, np.float32)
    k = np.asarray(k, np.float32)
    v = np.asarray(v, np.float32)
    mask_st = np.asarray(mask).reshape(S, S).astype(bool)

    cls, mtiles = _classify_mask(mask_st)
    with_bias = not (
        np.all(np.asarray(bq) == 0)
        and np.all(np.asarray(bk) == 0)
        and np.all(np.asarray(bv) == 0)
    )

    sig = (_cls_sig(cls), len(mtiles), with_bias)
    if sig not in _prog_cache:
        _prog_cache[sig] = _build(cls, len(mtiles), with_bias)
    nc = _prog_cache[sig]

    def pack_w(wt, gd):  # [nch*128, gd] -> [128, nch*gd]
        nch = wt.shape[0] // 128
        return np.ascontiguousarray(
            wt.reshape(nch, 128, gd).transpose(1, 0, 2).reshape(128, nch * gd)
        ).astype(_BF)

    in_maps = []
    for c in range(NCORE):
        b, g = divmod(c, TPG)
        rows = slice(g * GD, (g + 1) * GD)
        im = {
            "xqT": np.ascontiguousarray(q[b].T).astype(_BF),
            "xkT": np.ascontiguousarray(k[b].T).astype(_BF),
            "xvT": np.ascontiguousarray(v[b].T).astype(_BF),
            "WQ": pack_w(np.ascontiguousarray(Wq[rows, :].T), GD),
            "WK": pack_w(np.ascontiguousarray(Wk[rows, :].T), GD),
            "WV": pack_w(np.ascontiguousarray(Wv[rows, :].T), GD),
            "WO": pack_w(np.ascontiguousarray(Wo[:, rows].T), D),
        }
        if mtiles:
            im["MSK"] = np.stack(mtiles)
        if with_bias:
            im["BQ"] = np.asarray(bq)[rows].reshape(1, GD).astype(_BF)
            im["BK"] = np.asarray(bk)[rows].reshape(1, GD).astype(_BF)
            im["BV"] = np.asarray(bv)[rows].reshape(1, GD).astype(_BF)
        in_maps.append(im)

    res = run_bass_kernel_spmd(nc, in_maps, list(range(NCORE)), trace=TRACE)
    LAST_RESULT = res
    LAST_EXEC_NS = res.exec_time_ns

    out = np.zeros((B, S, D), np.float32)
    for c in range(NCORE):
        out[c // TPG] += res.results[c]["Y"]
    out += np.asarray(bo, np.float32)
    return out
